# revision 4
# baseline (speedup 1.0000x reference)
"""Fused dense-MLP kernel for Trainium2 (8 NeuronCores).

Computes: y = x @ W.T + b; GroupNorm(16 groups); SiLU; *mult_w; SiLU
Shapes: x [4096, 2048], W [8192, 2048], out [4096, 8192], fp32.

Strategy (hardcoded for these shapes):
- Tensor-parallel over out_features: each of the 8 cores owns 1024
  consecutive output features = 2 whole GroupNorm groups of 512, so the
  normalization statistics stay core-local.
- Host pre-transposes x and the W shard so both matmul operands arrive
  with the contraction dim (in_features) on partitions, and pre-rounds
  them to TF32 (dtype float32r) which the PE array processes at 4x the
  fp32 rate (1 cycle/row for free dim >= 256).
- Per core: W.T shard [2048, 1024] stays resident in SBUF; x.T is
  streamed in 32 batch tiles of [2048, 128]. Output tile y [128 batch,
  1024 out] is built in PSUM (2 accumulation groups of N=512 = one
  GroupNorm group each), then bias + GroupNorm stats (bn_stats/bn_aggr)
  + normalize + SiLU + mult_w + SiLU run on DVE/ACT while the PE works
  on the next tile.
"""

import numpy as np

B, IN_F, OUT_F, NG = 4096, 2048, 8192, 16
GS = OUT_F // NG  # 512, group size
N_CORES = 8
OUT_PC = OUT_F // N_CORES  # 1024 out features per core
G_PC = OUT_PC // GS  # 2 groups per core
KT = IN_F // 128  # 16 contraction tiles
MT = B // 128  # 32 batch tiles
EPS = 1e-5

_CACHE = {}


def _tf32_round(a: np.ndarray) -> np.ndarray:
    u = np.ascontiguousarray(a).view(np.uint32).astype(np.uint64)
    u = u + 0x0FFF + ((u >> 13) & 1)
    return (u & 0xFFFFE000).astype(np.uint32).view(np.float32)


def _build(mode: str, gn_affine: bool):
    import concourse.bacc as bacc
    import concourse.bass as bass
    import concourse.mybir as mybir
    import concourse.tile as tile

    FP = mybir.dt.float32
    mm_dt = {"fp32r": mybir.dt.float32r, "split3": mybir.dt.float32r, "fp32": FP}[mode]
    n_comp = 2 if mode == "split3" else 1

    nc = bacc.Bacc(None, target_bir_lowering=False)
    xT = nc.dram_tensor("xT", [n_comp, IN_F, B], mm_dt, kind="ExternalInput")
    wT = nc.dram_tensor("wT", [n_comp, IN_F, OUT_PC], mm_dt, kind="ExternalInput")
    vecs = nc.dram_tensor("vecs", [4, OUT_PC], FP, kind="ExternalInput")
    out = nc.dram_tensor("out", [B, OUT_PC], FP, kind="ExternalOutput")

    xT_r = xT.rearrange("c (k p) (mt mb) -> p c k mt mb", p=128, mb=128)
    wT_r = wT.rearrange("c (k p) n -> p c k n", p=128)

    if mode == "split3":
        passes = [(0, 0), (0, 1), (1, 0)]  # (x component, w component)
    else:
        passes = [(0, 0)]

    with tile.TileContext(nc) as tc:
        with (
            tc.tile_pool(name="wpool", bufs=1) as wpool,
            tc.tile_pool(name="xpool", bufs=3) as xpool,
            tc.tile_pool(name="ypool", bufs=3) as ypool,
            tc.tile_pool(name="spool", bufs=4) as spool,
            tc.tile_pool(name="cpool", bufs=1) as cpool,
            tc.tile_pool(name="psum", bufs=4, space="PSUM") as psum_pool,
        ):
            # --- constants: W.T shard resident; broadcast vectors ---
            wt_sb = wpool.tile([128, n_comp, KT, OUT_PC], mm_dt)
            for c in range(n_comp):
                for k in range(KT):
                    nc.sync.dma_start(out=wt_sb[:, c, k, :], in_=wT_r[:, c, k, :])

            def bcast_row(r):
                t = cpool.tile([128, OUT_PC], FP, tag=f"bc{r}")
                row = vecs[r : r + 1, :]
                ap = bass.AP(
                    tensor=row.tensor,
                    offset=row.offset,
                    ap=[[0, 128]] + list(row.ap)[1:],
                )
                nc.gpsimd.dma_start(out=t, in_=ap)
                return t

            b_bc = bcast_row(0)
            gnw_bc = bcast_row(1) if gn_affine else None
            gnb_bc = bcast_row(2) if gn_affine else None
            mw_bc = bcast_row(3)
            eps_t = cpool.tile([128, 1], FP, tag="eps")
            nc.vector.memset(eps_t, EPS)

            for m in range(MT):
                xt_m = xpool.tile([128, n_comp, KT, 128], mm_dt, tag="xt")
                nc.sync.dma_start(out=xt_m, in_=xT_r[:, :, :, m, :])

                y = ypool.tile([128, OUT_PC], FP, tag="y")
                st6 = spool.tile([128, G_PC, 6], FP, tag="st6")
                mv = spool.tile([128, G_PC, 2], FP, tag="mv")
                rstd = spool.tile([128, G_PC], FP, tag="rstd")
                nmean = spool.tile([128, G_PC], FP, tag="nmean")

                for g in range(G_PC):
                    gs = slice(g * GS, (g + 1) * GS)
                    ps = psum_pool.tile([128, GS], FP, tag="ps")
                    n_mm = len(passes) * KT
                    i = 0
                    for cx, cw in passes:
                        for k in range(KT):
                            nc.tensor.matmul(
                                ps,
                                xt_m[:, cx, k, :],
                                wt_sb[:, cw, k, gs],
                                start=(i == 0),
                                stop=(i == n_mm - 1),
                            )
                            i += 1
                    # evacuate PSUM with the bias add; stats on the result
                    nc.vector.tensor_add(out=y[:, gs], in0=ps, in1=b_bc[:, gs])
                    nc.vector.bn_stats(out=st6[:, g, :], in_=y[:, gs])
                    nc.vector.bn_aggr(out=mv[:, g, :], in_=st6[:, g, :])

                # rstd = 1/sqrt(var + eps); nmean = mean * rstd  (both groups)
                nc.scalar.activation(
                    out=rstd,
                    in_=mv[:, :, 1],
                    func=mybir.ActivationFunctionType.Sqrt,
                    bias=eps_t,
                )
                nc.vector.reciprocal(out=rstd, in_=rstd)
                nc.vector.tensor_mul(out=nmean, in0=mv[:, :, 0], in1=rstd)

                for g in range(G_PC):
                    gs = slice(g * GS, (g + 1) * GS)
                    # y = y * rstd - mean*rstd  == (y - mean) / std
                    nc.vector.tensor_scalar(
                        out=y[:, gs],
                        in0=y[:, gs],
                        scalar1=rstd[:, g : g + 1],
                        scalar2=nmean[:, g : g + 1],
                        op0=mybir.AluOpType.mult,
                        op1=mybir.AluOpType.subtract,
                    )
                if gn_affine:
                    nc.vector.tensor_mul(out=y, in0=y, in1=gnw_bc)
                    nc.vector.tensor_add(out=y, in0=y, in1=gnb_bc)

                nc.scalar.activation(
                    out=y, in_=y, func=mybir.ActivationFunctionType.Silu
                )
                nc.vector.tensor_mul(out=y, in0=y, in1=mw_bc)
                nc.scalar.activation(
                    out=y, in_=y, func=mybir.ActivationFunctionType.Silu
                )
                nc.sync.dma_start(out=out[m * 128 : (m + 1) * 128, :], in_=y)

    nc.compile()
    return nc


def _get_nc(mode: str, gn_affine: bool):
    key = (mode, gn_affine)
    if key not in _CACHE:
        _CACHE[key] = _build(mode, gn_affine)
    return _CACHE[key]


def _prep_mm(a: np.ndarray, mode: str) -> np.ndarray:
    """Host-side layout + rounding for a matmul operand ([..., K, N] f32)."""
    if mode == "fp32":
        return a[None]
    if mode == "fp32r":
        return _tf32_round(a)[None]
    hi = _tf32_round(a)
    lo = _tf32_round(a - hi)
    return np.stack([hi, lo])


def kernel(x, W, b, gn_w, gn_b, mult_w, mode="fp32r"):
    from concourse.bass_utils import run_bass_kernel_spmd

    x = np.ascontiguousarray(x, dtype=np.float32)
    W = np.ascontiguousarray(W, dtype=np.float32)
    b = np.asarray(b, dtype=np.float32)
    gn_w = np.asarray(gn_w, dtype=np.float32)
    gn_b = np.asarray(gn_b, dtype=np.float32)
    mult_w = np.asarray(mult_w, dtype=np.float32)

    gn_affine = not (
        np.all(gn_w == 1.0) and np.all(gn_b == 0.0)
    )
    nc = _get_nc(mode, gn_affine)

    xT = _prep_mm(np.ascontiguousarray(x.T), mode)  # [c, IN_F, B]
    in_maps = []
    for c in range(N_CORES):
        sl = slice(c * OUT_PC, (c + 1) * OUT_PC)
        wTc = _prep_mm(np.ascontiguousarray(W[sl].T), mode)  # [c, IN_F, OUT_PC]
        vecs = np.stack([b[sl], gn_w[sl], gn_b[sl], mult_w[sl]])
        in_maps.append({"xT": xT, "wT": wTc, "vecs": vecs})

    res = run_bass_kernel_spmd(nc, in_maps, list(range(N_CORES)))
    return np.concatenate([res.results[c]["out"] for c in range(N_CORES)], axis=1)


# revision 7
# speedup vs baseline: 3.9208x; 3.9208x over previous
"""Fused dense-MLP kernel for Trainium2 (8 NeuronCores).

Computes: y = x @ W.T + b; GroupNorm(16 groups); SiLU; *mult_w; SiLU
Shapes: x [4096, 2048], W [8192, 2048], out [4096, 8192], fp32.

Strategy (hardcoded for these shapes):
- Tensor-parallel over out_features: each of the 8 cores owns 1024
  consecutive output features = 2 whole GroupNorm groups of 512, so the
  normalization statistics stay core-local.
- Host pre-transposes x and the W shard so both matmul operands arrive
  with the contraction dim (in_features) on partitions, and pre-rounds
  them to TF32 (dtype float32r) which the PE array processes at 4x the
  fp32 rate (1 cycle/row for free dim >= 256).
- Per core: W.T shard [2048, 1024] stays resident in SBUF; x.T is
  streamed in 32 batch tiles of [2048, 128]. Output tile y [128 batch,
  1024 out] is built in PSUM (2 accumulation groups of N=512 = one
  GroupNorm group each), then bias + GroupNorm stats (bn_stats/bn_aggr)
  + normalize + SiLU + mult_w + SiLU run on DVE/ACT while the PE works
  on the next tile.
"""

import numpy as np

B, IN_F, OUT_F, NG = 4096, 2048, 8192, 16
GS = OUT_F // NG  # 512, group size
N_CORES = 8
OUT_PC = OUT_F // N_CORES  # 1024 out features per core
G_PC = OUT_PC // GS  # 2 groups per core
KT = IN_F // 128  # 16 contraction tiles
MT = B // 128  # 32 batch tiles
EPS = 1e-5

_CACHE = {}


def _tf32_round(a: np.ndarray) -> np.ndarray:
    u = np.ascontiguousarray(a).view(np.uint32).astype(np.uint64)
    u = u + 0x0FFF + ((u >> 13) & 1)
    return (u & 0xFFFFE000).astype(np.uint32).view(np.float32)


def _build(mode: str, gn_affine: bool, reps: int = 1):
    import concourse.bacc as bacc
    import concourse.bass as bass
    import concourse.mybir as mybir
    import concourse.tile as tile

    FP = mybir.dt.float32
    mm_dt = {"fp32r": mybir.dt.float32r, "split3": mybir.dt.float32r, "fp32": FP}[mode]
    n_comp = 2 if mode == "split3" else 1

    nc = bacc.Bacc(None, target_bir_lowering=False)
    xT = nc.dram_tensor("xT", [n_comp, IN_F, B], mm_dt, kind="ExternalInput")
    wT = nc.dram_tensor("wT", [n_comp, IN_F, OUT_PC], mm_dt, kind="ExternalInput")
    vecs = nc.dram_tensor("vecs", [4, OUT_PC], FP, kind="ExternalInput")
    out = nc.dram_tensor("out", [B, OUT_PC], FP, kind="ExternalOutput")

    xT_r = xT.rearrange("c (k p) (mt mb) -> p c k mt mb", p=128, mb=128)
    wT_r = wT.rearrange("c (k p) n -> p c k n", p=128)

    if mode == "split3":
        passes = [(0, 0), (0, 1), (1, 0)]  # (x component, w component)
    else:
        passes = [(0, 0)]

    with tile.TileContext(nc) as tc:
        with (
            tc.tile_pool(name="wpool", bufs=1) as wpool,
            tc.tile_pool(name="xpool", bufs=3) as xpool,
            tc.tile_pool(name="ypool", bufs=3) as ypool,
            tc.tile_pool(name="spool", bufs=4) as spool,
            tc.tile_pool(name="cpool", bufs=1) as cpool,
            tc.tile_pool(name="psum", bufs=4, space="PSUM") as psum_pool,
        ):
            # --- constants: W.T shard resident; broadcast vectors ---
            wt_sb = wpool.tile([128, n_comp, KT, OUT_PC], mm_dt)
            for c in range(n_comp):
                for k in range(KT):
                    nc.sync.dma_start(out=wt_sb[:, c, k, :], in_=wT_r[:, c, k, :])

            def bcast_row(r):
                t = cpool.tile([128, OUT_PC], FP, tag=f"bc{r}")
                row = vecs[r : r + 1, :]
                ap = bass.AP(
                    tensor=row.tensor,
                    offset=row.offset,
                    ap=[[0, 128]] + list(row.ap)[1:],
                )
                nc.gpsimd.dma_start(out=t, in_=ap)
                return t

            b_bc = bcast_row(0)
            gnw_bc = bcast_row(1) if gn_affine else None
            gnb_bc = bcast_row(2) if gn_affine else None
            mw_bc = bcast_row(3)
            eps_t = cpool.tile([128, 1], FP, tag="eps")
            nc.vector.memset(eps_t, EPS)

            for m in [mm for _ in range(reps) for mm in range(MT)]:
                xt_m = xpool.tile([128, n_comp, KT, 128], mm_dt, tag="xt")
                nc.sync.dma_start(out=xt_m, in_=xT_r[:, :, :, m, :])

                y = ypool.tile([128, OUT_PC], FP, tag="y")
                st6 = spool.tile([128, G_PC, 6], FP, tag="st6")
                mv = spool.tile([128, G_PC, 2], FP, tag="mv")
                rstd = spool.tile([128, G_PC], FP, tag="rstd")
                nmean = spool.tile([128, G_PC], FP, tag="nmean")

                for g in range(G_PC):
                    gs = slice(g * GS, (g + 1) * GS)
                    ps = psum_pool.tile([128, GS], FP, tag="ps")
                    n_mm = len(passes) * KT
                    i = 0
                    for cx, cw in passes:
                        for k in range(KT):
                            nc.tensor.matmul(
                                ps,
                                xt_m[:, cx, k, :],
                                wt_sb[:, cw, k, gs],
                                start=(i == 0),
                                stop=(i == n_mm - 1),
                            )
                            i += 1
                    # evacuate PSUM with the bias add; stats on the result
                    nc.vector.tensor_add(out=y[:, gs], in0=ps, in1=b_bc[:, gs])
                    nc.vector.bn_stats(out=st6[:, g, :], in_=y[:, gs])
                    nc.vector.bn_aggr(out=mv[:, g, :], in_=st6[:, g, :])

                # rstd = 1/sqrt(var + eps); nmean = mean * rstd  (both groups)
                nc.scalar.activation(
                    out=rstd,
                    in_=mv[:, :, 1],
                    func=mybir.ActivationFunctionType.Sqrt,
                    bias=eps_t,
                )
                nc.vector.reciprocal(out=rstd, in_=rstd)
                nc.vector.tensor_mul(out=nmean, in0=mv[:, :, 0], in1=rstd)

                for g in range(G_PC):
                    gs = slice(g * GS, (g + 1) * GS)
                    # y = y * rstd - mean*rstd  == (y - mean) / std
                    nc.vector.tensor_scalar(
                        out=y[:, gs],
                        in0=y[:, gs],
                        scalar1=rstd[:, g : g + 1],
                        scalar2=nmean[:, g : g + 1],
                        op0=mybir.AluOpType.mult,
                        op1=mybir.AluOpType.subtract,
                    )
                if gn_affine:
                    nc.vector.tensor_mul(out=y, in0=y, in1=gnw_bc)
                    nc.vector.tensor_add(out=y, in0=y, in1=gnb_bc)

                nc.scalar.activation(
                    out=y, in_=y, func=mybir.ActivationFunctionType.Silu
                )
                nc.vector.tensor_mul(out=y, in0=y, in1=mw_bc)
                nc.scalar.activation(
                    out=y, in_=y, func=mybir.ActivationFunctionType.Silu
                )
                nc.sync.dma_start(out=out[m * 128 : (m + 1) * 128, :], in_=y)

    nc.compile()
    return nc


def _get_nc(mode: str, gn_affine: bool, reps: int = 1):
    key = (mode, gn_affine, reps)
    if key not in _CACHE:
        _CACHE[key] = _build(mode, gn_affine, reps)
    return _CACHE[key]


def _prep_mm(a: np.ndarray, mode: str) -> np.ndarray:
    """Host-side layout + rounding for a matmul operand ([..., K, N] f32)."""
    if mode == "fp32":
        return a[None]
    if mode == "fp32r":
        return _tf32_round(a)[None]
    hi = _tf32_round(a)
    lo = _tf32_round(a - hi)
    return np.stack([hi, lo])


def kernel(x, W, b, gn_w, gn_b, mult_w, mode="fp32r"):
    from concourse.bass_utils import run_bass_kernel_spmd

    x = np.ascontiguousarray(x, dtype=np.float32)
    W = np.ascontiguousarray(W, dtype=np.float32)
    b = np.asarray(b, dtype=np.float32)
    gn_w = np.asarray(gn_w, dtype=np.float32)
    gn_b = np.asarray(gn_b, dtype=np.float32)
    mult_w = np.asarray(mult_w, dtype=np.float32)

    gn_affine = not (
        np.all(gn_w == 1.0) and np.all(gn_b == 0.0)
    )
    nc = _get_nc(mode, gn_affine)

    xT = _prep_mm(np.ascontiguousarray(x.T), mode)  # [c, IN_F, B]
    in_maps = []
    for c in range(N_CORES):
        sl = slice(c * OUT_PC, (c + 1) * OUT_PC)
        wTc = _prep_mm(np.ascontiguousarray(W[sl].T), mode)  # [c, IN_F, OUT_PC]
        vecs = np.stack([b[sl], gn_w[sl], gn_b[sl], mult_w[sl]])
        in_maps.append({"xT": xT, "wT": wTc, "vecs": vecs})

    res = run_bass_kernel_spmd(nc, in_maps, list(range(N_CORES)))
    return np.concatenate([res.results[c]["out"] for c in range(N_CORES)], axis=1)


# revision 20
# speedup vs baseline: 5.5070x; 1.4046x over previous
"""Fused dense-MLP kernel for Trainium2 (8 NeuronCores).

Computes: y = x @ W.T + b; GroupNorm(16 groups); SiLU; *mult_w; SiLU
Shapes: x [4096, 2048], W [8192, 2048], out [4096, 8192], fp32.

Strategy (hardcoded for these shapes):
- Tensor-parallel over out_features: each of the 8 cores owns 1024
  consecutive output features = 2 whole GroupNorm groups of 512, so the
  normalization statistics stay core-local.
- Host pre-transposes x and the W shard so both matmul operands arrive
  with the contraction dim (in_features) on partitions, and pre-rounds
  them to TF32 (dtype float32r) which the PE array processes at 4x the
  fp32 rate (1 cycle/row for free dim >= 256).
- Per core: W.T shard [2048, 1024] stays resident in SBUF; x.T is
  streamed in 32 batch tiles of [2048, 128]. Output tile y [128 batch,
  1024 out] is built in PSUM (2 accumulation groups of N=512 = one
  GroupNorm group each), then bias + GroupNorm stats (bn_stats/bn_aggr)
  + normalize + SiLU + mult_w + SiLU run on DVE/ACT while the PE works
  on the next tile.
"""

import numpy as np

B, IN_F, OUT_F, NG = 4096, 2048, 8192, 16
GS = OUT_F // NG  # 512, group size
N_CORES = 8
OUT_PC = OUT_F // N_CORES  # 1024 out features per core
G_PC = OUT_PC // GS  # 2 groups per core
KT = IN_F // 128  # 16 contraction tiles
MT = B // 128  # 32 batch tiles
EPS = 1e-5

_CACHE = {}


def _tf32_round(a: np.ndarray) -> np.ndarray:
    u = np.ascontiguousarray(a).view(np.uint32).astype(np.uint64)
    u = u + 0x0FFF + ((u >> 13) & 1)
    return (u & 0xFFFFE000).astype(np.uint32).view(np.float32)


def _build(
    mode: str,
    gn_affine: bool,
    reps: int = 1,
    psum_bufs: int = 4,
    x_bufs: int = 3,
    y_bufs: int = 3,
    mw_gpsimd: bool = False,
    k_inner: bool = False,
    no_xdma: bool = False,  # timing diagnostic only: wrong results
    no_epi: bool = False,  # timing diagnostic only: wrong results
    x_eng: str = "sync",  # which engine issues the x-tile loads
    x_batch: int = 1,  # batch tiles per x DMA
):
    import concourse.bacc as bacc
    import concourse.bass as bass
    import concourse.mybir as mybir
    import concourse.tile as tile

    FP = mybir.dt.float32
    mm_dt = {"fp32r": mybir.dt.float32r, "split3": mybir.dt.float32r, "fp32": FP}[mode]
    n_comp = 2 if mode == "split3" else 1

    nc = bacc.Bacc(None, target_bir_lowering=False)
    xT = nc.dram_tensor("xT", [n_comp, IN_F, B], mm_dt, kind="ExternalInput")
    wT = nc.dram_tensor("wT", [n_comp, IN_F, OUT_PC], mm_dt, kind="ExternalInput")
    vecs = nc.dram_tensor("vecs", [4, OUT_PC], FP, kind="ExternalInput")
    out = nc.dram_tensor("out", [B, OUT_PC], FP, kind="ExternalOutput")

    xT_r = xT.rearrange("c (k p) (mt mb) -> p c k mt mb", p=128, mb=128)
    wT_r = wT.rearrange("c (k p) n -> p c k n", p=128)

    if mode == "split3":
        passes = [(0, 0), (0, 1), (1, 0)]  # (x component, w component)
    else:
        passes = [(0, 0)]

    with tile.TileContext(nc) as tc:
        with (
            tc.tile_pool(name="wpool", bufs=1) as wpool,
            tc.tile_pool(name="xpool", bufs=x_bufs) as xpool,
            tc.tile_pool(name="ypool", bufs=y_bufs) as ypool,
            tc.tile_pool(name="spool", bufs=4) as spool,
            tc.tile_pool(name="cpool", bufs=1) as cpool,
            tc.tile_pool(name="psum", bufs=psum_bufs, space="PSUM") as psum_pool,
        ):
            # --- constants: W.T shard resident; broadcast vectors ---
            wt_sb = wpool.tile([128, n_comp, KT, OUT_PC], mm_dt)
            for c in range(n_comp):
                for k in range(KT):
                    nc.sync.dma_start(out=wt_sb[:, c, k, :], in_=wT_r[:, c, k, :])

            def bcast_row(r):
                t = cpool.tile([128, OUT_PC], FP, tag=f"bc{r}")
                row = vecs[r : r + 1, :]
                ap = bass.AP(
                    tensor=row.tensor,
                    offset=row.offset,
                    ap=[[0, 128]] + list(row.ap)[1:],
                )
                nc.gpsimd.dma_start(out=t, in_=ap)
                return t

            b_bc = bcast_row(0)
            gnw_bc = bcast_row(1) if gn_affine else None
            gnb_bc = bcast_row(2) if gn_affine else None
            mw_bc = bcast_row(3)
            eps_t = cpool.tile([128, 1], FP, tag="eps")
            nc.vector.memset(eps_t, EPS)

            x_dma_eng = {"sync": nc.sync, "scalar": nc.scalar, "gpsimd": nc.gpsimd}[
                x_eng
            ]
            xt_shared = None
            if no_xdma:
                xt_shared = xpool.tile([128, n_comp, KT, 128], mm_dt, tag="xt")
                x_dma_eng.dma_start(out=xt_shared, in_=xT_r[:, :, :, 0, :])
            xt_cur = None
            for m in [mm for _ in range(reps) for mm in range(MT)]:
                if no_xdma:
                    xt_m = xt_shared
                elif x_batch > 1:
                    mb = m % x_batch
                    if mb == 0:
                        xt_cur = xpool.tile(
                            [128, n_comp, KT, x_batch, 128], mm_dt, tag="xt"
                        )
                        x_dma_eng.dma_start(
                            out=xt_cur, in_=xT_r[:, :, :, m : m + x_batch, :]
                        )
                    xt_m = xt_cur[:, :, :, mb, :]
                else:
                    xt_m = xpool.tile([128, n_comp, KT, 128], mm_dt, tag="xt")
                    x_dma_eng.dma_start(out=xt_m, in_=xT_r[:, :, :, m, :])

                y = ypool.tile([128, OUT_PC], FP, tag="y")
                st6 = spool.tile([128, G_PC, 6], FP, tag="st6")
                mv = spool.tile([128, G_PC, 2], FP, tag="mv")
                rstd = spool.tile([128, G_PC], FP, tag="rstd")
                nmean = spool.tile([128, G_PC], FP, tag="nmean")

                n_mm = len(passes) * KT
                if k_inner:
                    # Both groups' accumulations open at once; consecutive
                    # matmuls share the same stationary operand (xt k-tile).
                    ps0 = psum_pool.tile([128, GS], FP, tag="ps0")
                    ps1 = psum_pool.tile([128, GS], FP, tag="ps1")
                    pss = [ps0, ps1]
                    i = 0
                    for cx, cw in passes:
                        for k in range(KT):
                            for g in range(G_PC):
                                gs = slice(g * GS, (g + 1) * GS)
                                nc.tensor.matmul(
                                    pss[g],
                                    xt_m[:, cx, k, :],
                                    wt_sb[:, cw, k, gs],
                                    start=(i == 0),
                                    stop=(i == n_mm - 1),
                                )
                            i += 1
                    for g in range(G_PC):
                        gs = slice(g * GS, (g + 1) * GS)
                        nc.vector.tensor_add(out=y[:, gs], in0=pss[g], in1=b_bc[:, gs])
                        nc.vector.bn_stats(out=st6[:, g, :], in_=y[:, gs])
                        nc.vector.bn_aggr(out=mv[:, g, :], in_=st6[:, g, :])
                else:
                    for g in range(G_PC):
                        gs = slice(g * GS, (g + 1) * GS)
                        ps = psum_pool.tile([128, GS], FP, tag="ps")
                        i = 0
                        for cx, cw in passes:
                            for k in range(KT):
                                nc.tensor.matmul(
                                    ps,
                                    xt_m[:, cx, k, :],
                                    wt_sb[:, cw, k, gs],
                                    start=(i == 0),
                                    stop=(i == n_mm - 1),
                                )
                                i += 1
                        # evacuate PSUM with the bias add; stats on the result
                        nc.vector.tensor_add(out=y[:, gs], in0=ps, in1=b_bc[:, gs])
                        if not no_epi:
                            nc.vector.bn_stats(out=st6[:, g, :], in_=y[:, gs])
                            nc.vector.bn_aggr(out=mv[:, g, :], in_=st6[:, g, :])

                if no_epi:
                    nc.sync.dma_start(out=out[m * 128 : (m + 1) * 128, :], in_=y)
                    continue
                # rstd = 1/sqrt(var + eps); nmean = mean * rstd  (both groups)
                nc.scalar.activation(
                    out=rstd,
                    in_=mv[:, :, 1],
                    func=mybir.ActivationFunctionType.Sqrt,
                    bias=eps_t,
                )
                nc.vector.reciprocal(out=rstd, in_=rstd)
                nc.vector.tensor_mul(out=nmean, in0=mv[:, :, 0], in1=rstd)

                for g in range(G_PC):
                    gs = slice(g * GS, (g + 1) * GS)
                    # y = y * rstd - mean*rstd  == (y - mean) / std
                    nc.vector.tensor_scalar(
                        out=y[:, gs],
                        in0=y[:, gs],
                        scalar1=rstd[:, g : g + 1],
                        scalar2=nmean[:, g : g + 1],
                        op0=mybir.AluOpType.mult,
                        op1=mybir.AluOpType.subtract,
                    )
                if gn_affine:
                    nc.vector.tensor_mul(out=y, in0=y, in1=gnw_bc)
                    nc.vector.tensor_add(out=y, in0=y, in1=gnb_bc)

                nc.scalar.activation(
                    out=y, in_=y, func=mybir.ActivationFunctionType.Silu
                )
                if mw_gpsimd:
                    nc.gpsimd.tensor_mul(out=y, in0=y, in1=mw_bc)
                else:
                    nc.vector.tensor_mul(out=y, in0=y, in1=mw_bc)
                nc.scalar.activation(
                    out=y, in_=y, func=mybir.ActivationFunctionType.Silu
                )
                nc.sync.dma_start(out=out[m * 128 : (m + 1) * 128, :], in_=y)

    nc.compile()
    return nc


def _get_nc(mode: str, gn_affine: bool, reps: int = 1, **opts):
    key = (mode, gn_affine, reps, tuple(sorted(opts.items())))
    if key not in _CACHE:
        _CACHE[key] = _build(mode, gn_affine, reps, **opts)
    return _CACHE[key]


def _prep_mm(a: np.ndarray, mode: str) -> np.ndarray:
    """Host-side layout + rounding for a matmul operand ([..., K, N] f32)."""
    if mode == "fp32":
        return a[None]
    if mode == "fp32r":
        return _tf32_round(a)[None]
    hi = _tf32_round(a)
    lo = _tf32_round(a - hi)
    return np.stack([hi, lo])


def kernel(x, W, b, gn_w, gn_b, mult_w, mode="fp32r"):
    from concourse.bass_utils import run_bass_kernel_spmd

    x = np.ascontiguousarray(x, dtype=np.float32)
    W = np.ascontiguousarray(W, dtype=np.float32)
    b = np.asarray(b, dtype=np.float32)
    gn_w = np.asarray(gn_w, dtype=np.float32)
    gn_b = np.asarray(gn_b, dtype=np.float32)
    mult_w = np.asarray(mult_w, dtype=np.float32)

    gn_affine = not (
        np.all(gn_w == 1.0) and np.all(gn_b == 0.0)
    )
    nc = _get_nc(mode, gn_affine)

    xT = _prep_mm(np.ascontiguousarray(x.T), mode)  # [c, IN_F, B]
    in_maps = []
    for c in range(N_CORES):
        sl = slice(c * OUT_PC, (c + 1) * OUT_PC)
        wTc = _prep_mm(np.ascontiguousarray(W[sl].T), mode)  # [c, IN_F, OUT_PC]
        vecs = np.stack([b[sl], gn_w[sl], gn_b[sl], mult_w[sl]])
        in_maps.append({"xT": xT, "wT": wTc, "vecs": vecs})

    res = run_bass_kernel_spmd(nc, in_maps, list(range(N_CORES)))
    return np.concatenate([res.results[c]["out"] for c in range(N_CORES)], axis=1)
